# revision 2
# baseline (speedup 1.0000x reference)
"""Bilinear RGGB demosaic (Bayer -> RGB) on 8 Trainium2 NeuronCores.

Layout: batch image n -> core n. Per core, the [2048, 2048] mosaic is
processed in 8 bands of 256 rows; SBUF partition p of a band holds the
row pair (2p, 2p+1) concatenated in the free dim, so every DRAM transfer
is 16 KiB-contiguous per partition. Column-direction stencil taps are
free-dim shifted APs; the two row-direction taps are produced on the
tensor engine as 0.25*(row_{p-1}+row_p) / 0.25*(row_p+row_{p+1}) via
banded 128x128 fp32 matmuls (band-seam rows fixed up with K=2 matmuls
against a 2-row halo tile). VectorE assembles the averaged phases,
ScalarE the pass-through/2x phases, GpSimd the two 0.5x phases.
"""

import sys

sys.path.insert(0, "/opt/trn_rl_repo")

import numpy as np

import concourse.bass as bass
import concourse.tile as tile
from concourse import mybir
from concourse.alu_op_type import AluOpType
from concourse.bass_utils import run_bass_kernel_spmd

F32 = mybir.dt.float32
MM_DT = mybir.dt.float32  # matmul operand dtype (float32r = fast PE mode)
N_CORES = 8
H = 2048
W = 2048
N_BANDS = H // 256  # 128 row-pairs per band


def split_sync_waits(nc, max_waits=1):
    """This walrus build rejects instructions carrying more than
    `max_waits` sync-wait commands. Hoist excess waits onto same-engine
    NoOps inserted immediately before the over-subscribed instruction
    (waiting earlier on the same queue is semantically conservative)."""
    for fn in nc.m.functions:
        for bb in fn.blocks:
            insts = bb.instructions
            i = 0
            while i < len(insts):
                inst = insts[i]
                si = inst.sync_info
                waits = list(si.on_wait) if si and si.on_wait else []
                if len(waits) > max_waits:
                    si.on_wait = waits[:max_waits]
                    excess = waits[max_waits:]
                    for j in range(0, len(excess), max_waits):
                        nop = mybir.InstNoOp(
                            name=nc.get_next_instruction_name(), ins=[], outs=[]
                        )
                        nop.engine = inst.engine
                        nop.sync_info = mybir.SyncInfo(
                            on_wait=excess[j : j + max_waits], on_update=[]
                        )
                        nc.register_instruction(nop)
                        insts.insert(i, nop)
                        i += 1
                i += 1


def const_arrays():
    m1 = 0.25 * (np.eye(128, dtype=np.float32) + np.eye(128, k=1, dtype=np.float32))
    m2 = 0.25 * (np.eye(128, dtype=np.float32) + np.eye(128, k=-1, dtype=np.float32))
    cmm = np.concatenate([m1, m2], axis=1)  # [128, 256]
    cfx = np.zeros((2, 256), dtype=np.float32)
    cfx[0, 0] = 0.25  # fu: Su4[0] += 0.25 * prevO   (halo row 0)
    cfx[1, 128 + 127] = 0.25  # fd: Sd4[127] += 0.25 * nextE (halo row 1)
    return cmm, cfx


def band_plan(npairs):
    """Bands of 128 row-pairs advancing ~126 pairs: each band stores only
    the pair range whose vertical neighbors are in-tile, so no halo or
    seam-fix work is needed. Returns [(start_pair, store_lo, store_hi)]."""
    plan = []
    covered = 0
    while covered < npairs:
        q = 0 if covered == 0 else min(covered - 1, npairs - 128)
        lo = covered - q
        hi = 128 if q + 128 >= npairs else 127
        plan.append((q, lo, hi))
        covered = q + hi
    return plan


def build_program(npairs=H // 2, w=W, repeats=1, variant="full"):
    nc = bass.Bass("TRN2", target_bir_lowering=False, debug=False)
    x = nc.dram_tensor("x", [npairs, 2 * w], F32, kind="ExternalInput").ap()
    cmm = nc.dram_tensor("cmm", [128, 256], F32, kind="ExternalInput").ap()
    out = nc.dram_tensor("out", [3, npairs, 2 * w], F32, kind="ExternalOutput").ap()

    cw = min(512, w)  # matmul free-dim chunk (PSUM bank)
    hw = w // 2
    plan = band_plan(npairs)

    with tile.TileContext(nc) as tc:
        with (
            tc.tile_pool(name="consts", bufs=1) as cpool,
            tc.tile_pool(name="inp", bufs=3) as ipool,
            tc.tile_pool(name="psum", bufs=2, space="PSUM") as ppool,
            tc.tile_pool(name="mids", bufs=2) as mpool,
            tc.tile_pool(name="tmps", bufs=1) as tpool,
            tc.tile_pool(name="outs", bufs=2) as opool,
        ):
            cM = cpool.tile([128, 256], F32)
            nc.sync.dma_start(cM[:], cmm[:])

            def body():
                for q, lo, hi in plan:
                    IN = ipool.tile([128, 2 * w], F32, tag="in")
                    if variant != "nodma":
                        nc.sync.dma_start(IN[:], x[q : q + 128, :])
                    else:
                        nc.gpsimd.memset(IN[:, 0:2], 0.0)

                    E = IN[:, 0:w]
                    O = IN[:, w : 2 * w]

                    # Su4[p] = 0.25*(O[p-1] + O[p]) ; Sd4[p] = 0.25*(E[p] + E[p+1])
                    # PSUM chunked at half-band width, double-buffered.
                    Su4 = mpool.tile([128, w], F32, tag="su_sb")
                    Sd4 = mpool.tile([128, w], F32, tag="sd_sb")
                    pw = min(2 * cw, w)
                    mm = lambda ap: ap.bitcast(MM_DT)
                    for h0 in range(0, w, pw):
                        Su4p = ppool.tile([128, pw], F32, tag="su")
                        Sd4p = ppool.tile([128, pw], F32, tag="sd")
                        for c in range(0, pw, cw):
                            s = slice(h0 + c, h0 + c + cw)
                            sp = slice(c, c + cw)
                            nc.tensor.matmul(
                                Su4p[:, sp], mm(cM[:, 0:128]), mm(O[:, s]),
                                start=True, stop=True,
                            )
                            nc.tensor.matmul(
                                Sd4p[:, sp], mm(cM[:, 128:256]), mm(E[:, s]),
                                start=True, stop=True,
                            )
                        nc.scalar.copy(Su4[:, h0 : h0 + pw], Su4p[:])
                        nc.scalar.copy(Sd4[:, h0 : h0 + pw], Sd4p[:])

                    Rt = opool.tile([128, 2 * w], F32, tag="r")
                    Gt = opool.tile([128, 2 * w], F32, tag="g")
                    Bt = opool.tile([128, 2 * w], F32, tag="b")

                    # ---- R channel ----
                    # even rows, even cols: passthrough E
                    nc.scalar.copy(Rt[:, 0:w:2], E[:, 0:w:2])
                    # even rows, odd cols: 0.5*(E[x-1] + E[x+1])
                    te = tpool.tile([128, hw], F32, tag="te")
                    nc.vector.tensor_add(te[:, 0 : hw - 1], E[:, 0 : w - 2 : 2], E[:, 2:w:2])
                    (nc.scalar.mul if variant == "noGp" else lambda o, i, c: nc.gpsimd.tensor_scalar_mul(o, i, c))(Rt[:, 1 : w - 2 : 2], te[:, 0 : hw - 1], 0.5)
                    nc.vector.tensor_scalar_mul(
                        Rt[:, w - 1 : w], E[:, w - 2 : w - 1], 0.5
                    )
                    # odd rows, even cols: 2*Sd4
                    nc.scalar.mul(Rt[:, w : 2 * w : 2], Sd4[:, 0:w:2], 2.0)
                    # odd rows, odd cols: Sd4[x-1] + Sd4[x+1]
                    nc.vector.tensor_add(
                        Rt[:, w + 1 : 2 * w - 2 : 2], Sd4[:, 0 : w - 2 : 2], Sd4[:, 2:w:2]
                    )
                    nc.vector.tensor_copy(
                        Rt[:, 2 * w - 1 : 2 * w], Sd4[:, w - 2 : w - 1]
                    )

                    # ---- G channel ----
                    # even rows, even cols: 0.25*(E[x-1]+E[x+1]) + Su4[x]
                    tg = tpool.tile([128, hw], F32, tag="tg")
                    nc.vector.tensor_add(tg[:, 0 : hw - 1], E[:, 1 : w - 2 : 2], E[:, 3:w:2])
                    nc.vector.scalar_tensor_tensor(
                        Gt[:, 2 : w - 1 : 2], tg[:, 0 : hw - 1], 0.25,
                        Su4[:, 2 : w - 1 : 2], AluOpType.mult, AluOpType.add,
                    )
                    nc.vector.scalar_tensor_tensor(
                        Gt[:, 0:1], E[:, 1:2], 0.25, Su4[:, 0:1],
                        AluOpType.mult, AluOpType.add,
                    )
                    # even rows, odd cols: passthrough E
                    nc.scalar.copy(Gt[:, 1:w:2], E[:, 1:w:2])
                    # odd rows, even cols: passthrough O
                    nc.scalar.copy(Gt[:, w : 2 * w : 2], O[:, 0:w:2])
                    # odd rows, odd cols: 0.25*(O[x-1]+O[x+1]) + Sd4[x]
                    tg2 = tpool.tile([128, hw], F32, tag="tg2")
                    nc.vector.tensor_add(
                        tg2[:, 0 : hw - 1], O[:, 0 : w - 2 : 2], O[:, 2:w:2]
                    )
                    nc.vector.scalar_tensor_tensor(
                        Gt[:, w + 1 : 2 * w - 2 : 2], tg2[:, 0 : hw - 1], 0.25,
                        Sd4[:, 1 : w - 2 : 2], AluOpType.mult, AluOpType.add,
                    )
                    nc.vector.scalar_tensor_tensor(
                        Gt[:, 2 * w - 1 : 2 * w], O[:, w - 2 : w - 1], 0.25,
                        Sd4[:, w - 1 : w], AluOpType.mult, AluOpType.add,
                    )

                    # ---- B channel ----
                    # even rows, even cols: Su4[x-1] + Su4[x+1]
                    nc.vector.tensor_add(
                        Bt[:, 2 : w - 1 : 2], Su4[:, 1 : w - 2 : 2], Su4[:, 3:w:2]
                    )
                    nc.vector.tensor_copy(Bt[:, 0:1], Su4[:, 1:2])
                    # even rows, odd cols: 2*Su4
                    nc.scalar.mul(Bt[:, 1:w:2], Su4[:, 1:w:2], 2.0)
                    # odd rows, even cols: 0.5*(O[x-1]+O[x+1])
                    tb = tpool.tile([128, hw], F32, tag="tb")
                    nc.vector.tensor_add(tb[:, 0 : hw - 1], O[:, 1 : w - 2 : 2], O[:, 3:w:2])
                    (nc.scalar.mul if variant == "noGp" else lambda o, i, c: nc.gpsimd.tensor_scalar_mul(o, i, c))(
                        Bt[:, w + 2 : 2 * w - 1 : 2], tb[:, 0 : hw - 1], 0.5
                    )
                    nc.vector.tensor_scalar_mul(Bt[:, w : w + 1], O[:, 1:2], 0.5)
                    # odd rows, odd cols: passthrough O
                    nc.scalar.copy(Bt[:, w + 1 : 2 * w : 2], O[:, 1:w:2])

                    if variant != "nodma":
                        for c in range(3):
                            nc.sync.dma_start(
                                out[c, q + lo : q + hi, :], [Rt, Gt, Bt][c][lo:hi, :]
                            )

            if repeats == 1:
                body()
            else:
                with tc.For_i(0, repeats, 1):
                    body()

    split_sync_waits(nc)
    return nc


_CACHE = {}

TRACE = False
LAST_RESULT = None


def _get_program(npairs, w):
    key = (npairs, w)
    if key not in _CACHE:
        _CACHE[key] = build_program(npairs, w)
    return _CACHE[key]


def kernel(x: np.ndarray) -> np.ndarray:
    global LAST_RESULT
    n, _, h, w = x.shape
    assert (n, h, w) == (N_CORES, H, W), x.shape
    nc = _get_program(H // 2, W)
    cmm, _ = const_arrays()
    in_maps = []
    for i in range(N_CORES):
        img = np.ascontiguousarray(x[i, 0], dtype=np.float32).reshape(H // 2, 2 * W)
        in_maps.append({"x": img, "cmm": cmm})
    res = run_bass_kernel_spmd(
        nc, in_maps, core_ids=list(range(N_CORES)), trace=TRACE
    )
    LAST_RESULT = res
    outs = [res.results[i]["out"].reshape(3, H, W)[None] for i in range(N_CORES)]
    return np.concatenate(outs, axis=0)

